# revision 24
# baseline (speedup 1.0000x reference)
"""MoE-routed DeepQNetwork kernel for 8x Trainium2 NeuronCores.

Problem: B=65536 rows, each routed to one of E=8 expert MLPs
(256 -> 64 -> 64 -> 64 -> 64 -> 64 -> 18, ReLU between layers).

Strategy v6 (expert-per-core sharding, software-pipelined wavefront):
  E == NCORES and the routing is near-uniform (~8192 rows/expert), so core k
  owns ALL rows of expert k, padded to a uniform C = nb*512 columns (nb may
  be odd: pairs of 512-row blocks plus one lone block). Every core runs the
  same static program with a SINGLE expert's weights (~180 KB).

  Device (per core, SPMD):
  - A short burst of tiny matmuls right after the preamble releases the HAM
    clock gate (the PE idles at 1.2 GHz until ~3.4us of sustained activity)
    so real matmuls run at 2.4 GHz from the start.
  - x^T arrives as [256, C] fp16 on the sync HW-DGE ring in consumption
    order (pair0 halves, pair1, then 1MB two-pair chunks, lone block last);
    weights+bias ride the scalar HW-DGE ring concurrently. A single ordered
    ring measured fastest (splitting x across rings halved early bandwidth).
  - Compute is a depth-6 software pipeline over "units" (two-pair groups +
    the lone block): each wave emits L6/L5/L4/L3/L2 of progressively older
    units before L1 of the newest, so the PE fills x-DMA wait time with
    deeper-layer work and output stores spread across the whole run instead
    of bunching in a serial tail.
  - L1 per pair: 4 matmuls on PE column groups (block even -> PSUM rows
    0:64, odd -> 64:128), contraction 256 over two accumulating chunks,
    per-pair ReLU+bias. L2-5: [128,128] block-diag matmuls per pair (same
    64x64 weight on both diagonals) into two-pair [128,1024] PSUM tiles
    drained by one ReLU+bias per two pairs. L6 stacks two pairs into one
    [128,512] PSUM bank (rows 0:18/32:50/64:82/96:114), bias-added and
    stored as fp16 via gpsimd-issued DMAs.

  Host: unsort the fp16 outputs back to row order, cast to fp32.
"""

import math
import os

import numpy as np

E = 8
D = 256
H = 64
A = 18
NCORES = 8
BLK = 512  # rows per block (matmul moving-operand free dim / PSUM bank cols)
NWARM = 4  # PE warm-up matmuls bridging preamble-end to first x arrival

# per-core weight tile [128, WCOLS] fp16 column layout:
#   [0:64)    W1 chunk0 (input dims 0:128)
#   [64:128)  W1 chunk1 (input dims 128:256)
#   [128+128*li : 256+128*li) for li in 0..3: layer 2+li block-diag [128,128]
#             ([0:64,0:64] = W, [64:128,64:128] = W)
#   [640:704) W6 block-diag: [0:64, 0:18] = W6, [64:128, 32:50] = W6
WCOLS = 704

_PROGRAM_CACHE: dict = {}
LAST_RESULTS = None  # test harness can read timing/profile info from here


def _build_program(nb: int):
    """Build the SPMD bass program for nb 512-row blocks per core."""
    import concourse.mybir as mybir
    import concourse.tile as tile
    from concourse import bacc

    f32 = mybir.dt.float32
    f16 = mybir.dt.float16
    Relu = mybir.ActivationFunctionType.Relu
    add = mybir.AluOpType.add
    amax = mybir.AluOpType.max

    npair = nb // 2
    lone = nb % 2  # trailing unpaired block
    ndbl = (npair + 1) // 2  # two-pair groups (last may hold one pair)
    ngrp = ndbl + lone  # output column groups in yt

    nc = bacc.Bacc("TRN2")
    xall = nc.declare_dram_parameter(
        "xall", [128, npair * 2048 + lone * 1024], f16, isOutput=False
    )
    wt = nc.declare_dram_parameter("wt", [128, WCOLS], f16, isOutput=False)
    # bias cols 0:5 = b1..b5 (rows 0:64 == rows 64:128); col 5 = b6 at rows
    # 0:18 / 32:50 / 64:82 / 96:114
    bias = nc.declare_dram_parameter("bias", [128, 6], f32, isOutput=False)
    # output: group g (two pairs side by side) at cols [g*1024, g*1024+1024);
    # within a pair's 512 cols, block even at rows 0:18, block odd at 32:50.
    # 64-partition x 2KB rows DMA efficiently; the lone block uses the first
    # 512 cols of its own trailing group.
    yt = nc.declare_dram_parameter(
        "yt", [64, ndbl * 2 * BLK + lone * BLK], f16, isOutput=True
    )

    act_flip = 0

    with tile.TileContext(nc) as tc:
        with (
            tc.tile_pool(name="wpool", bufs=1) as wpool,
            tc.tile_pool(name="xpool", bufs=2) as xpool,
            tc.tile_pool(name="hpool", bufs=2) as hpool,
            tc.tile_pool(name="opool", bufs=3) as opool,
            tc.tile_pool(name="ppool", bufs=3, space="PSUM") as ppool,
            tc.tile_pool(name="popool", bufs=1, space="PSUM") as popool,
        ):
            def dbl_pairs(d):
                return [q for q in (2 * d, 2 * d + 1) if q < npair]

            # ---- PE warm-up source (memset, no DMA dependence)
            warm_src = wpool.tile([1, BLK], f16, name="warm_src", tag="ws", bufs=1)
            nc.vector.memset(warm_src[:, :], 0.0)

            # ---- DMA issue. All x on the sync HW-DGE ring in consumption
            # order; weights+bias on the scalar HW-DGE ring; outputs on
            # gpsimd. (Splitting x across rings measured slower.)
            w_sb = wpool.tile([128, WCOLS], f16, name="w_sb", tag="w", bufs=1)
            nc.scalar.dma_start(out=w_sb[:, :], in_=wt[:, :])
            bias_sb = wpool.tile([128, 6], f32, name="bias_sb", tag="bias", bufs=1)
            nc.scalar.dma_start(out=bias_sb[:, :], in_=bias[:, :])

            # pair0 as two [128,1024] half-chunks (first-needed first), then
            # pair1, then 1MB two-pair chunks; the lone block arrives last
            # (it is also processed last, so the pipeline drain is short).
            p0 = []
            for i in (0, 1):
                t = xpool.tile([128, 1024], f16, tag=f"x0h{i}", name=f"x0h{i}", bufs=1)
                nc.sync.dma_start(out=t[:, :], in_=xall[:, i * 1024 : (i + 1) * 1024])
                p0.append(t)
            xc1 = None
            if npair > 1:
                xc1 = xpool.tile([128, 2048], f16, tag="xc1", name="xc_1", bufs=1)
                nc.sync.dma_start(out=xc1[:, :], in_=xall[:, 2048:4096])
            xds: list = [None] * ndbl
            for dd in range(1, ndbl):
                w = len(dbl_pairs(dd)) * 2048
                xd = xpool.tile([128, w], f16, tag=f"xd{dd}", name=f"xd_{dd}", bufs=1)
                nc.sync.dma_start(
                    out=xd[:, :], in_=xall[:, 2 * dd * 2048 : 2 * dd * 2048 + w]
                )
                xds[dd] = xd
            xl = None
            if lone:
                xl = xpool.tile([128, 1024], f16, tag="xl", name="xlone", bufs=1)
                nc.sync.dma_start(
                    out=xl[:, :], in_=xall[:, npair * 2048 : npair * 2048 + 1024]
                )

            def x_rhs(p, blk, c):
                if p == 0:
                    return p0[c][:, blk * BLK : (blk + 1) * BLK]
                if p == 1:
                    return xc1[:, c * 1024 + blk * BLK : c * 1024 + (blk + 1) * BLK]
                xd = xds[p // 2]
                off = (p % 2) * 2048 + c * 1024 + blk * BLK
                return xd[:, off : off + BLK]

            # ---- PE warm-up burst (writes cycle the ph ring, never read)
            for i in range(NWARM):
                pw = ppool.tile([128, 1024], f32, tag="ph", name=f"warm_{i}")
                nc.tensor.matmul(
                    out=pw[0:64, 0:BLK],
                    lhsT=warm_src[0:1, 0:64],
                    rhs=warm_src[0:1, :],
                    start=True,
                    stop=True,
                )

            eng_ns = [0.0, 0.0]  # cumulative vector / scalar act cost

            def act(out_ap, in_ap, bias_ap, relu):
                # greedy engine balance on the errata-adjusted cost model
                fd = in_ap.shape[-1]
                vc = (120 + fd) / 0.96
                sc = (352 + fd) / 1.2
                if eng_ns[0] + vc <= eng_ns[1] + sc:
                    eng_ns[0] += vc
                    if relu:
                        nc.vector.tensor_scalar(
                            out_ap, in_ap, bias_ap, 0.0, op0=add, op1=amax
                        )
                    else:
                        nc.vector.tensor_scalar(out_ap, in_ap, bias_ap, None, op0=add)
                else:
                    eng_ns[1] += sc
                    if relu:
                        nc.scalar.activation(out_ap, in_ap, Relu, bias=bias_ap)
                    else:
                        nc.scalar.add(out_ap, in_ap, bias_ap)

            # h storage: layer 1 per pair (+ lone), layers 2-5 per dbl (+ lone)
            h1s = [None] * npair
            hdbl = {li: [None] * ndbl for li in (2, 3, 4, 5)}
            hlon = {}

            def emit_s1(u):
                if u == -1:
                    phl = ppool.tile([128, 1024], f32, tag="ph", name="ph1_l")
                    for c in (0, 1):
                        nc.tensor.matmul(
                            out=phl[0:64, 0:BLK],
                            lhsT=w_sb[:, c * H : (c + 1) * H],
                            rhs=xl[:, c * BLK : (c + 1) * BLK],
                            start=(c == 0),
                            stop=(c == 1),
                        )
                    hl = hpool.tile([64, BLK], f16, tag="hl1", name="h1_l", bufs=1)
                    act(hl[:, :], phl[0:64, 0:BLK], bias_sb[0:64, 0:1], True)
                    hlon[1] = hl
                    return
                ph1 = ppool.tile([128, 1024], f32, tag="ph", name=f"ph1_{u}")
                for k, p in enumerate(dbl_pairs(u)):
                    co = k * BLK
                    for blk, colr in ((0, slice(0, 64)), (1, slice(64, 128))):
                        for c in (0, 1):
                            nc.tensor.matmul(
                                out=ph1[colr, co : co + BLK],
                                lhsT=w_sb[:, c * H : (c + 1) * H],
                                rhs=x_rhs(p, blk, c),
                                start=(c == 0),
                                stop=(c == 1),
                            )
                    # per-pair activation: finer PSUM-drain granularity in
                    # the x-DMA-paced phase
                    h1 = hpool.tile(
                        [128, BLK], f16, tag=f"h1_{p}", name=f"h1_{p}", bufs=1
                    )
                    act(h1[:, :], ph1[:, co : co + BLK], bias_sb[:, 0:1], True)
                    h1s[p] = h1

            def emit_mid(li, u):
                # layer li in 2..5: [64 -> 64] block-diag
                wc = 128 + (li - 2) * 128
                bap_rows = slice(0, 64)
                if u == -1:
                    prev = hlon[1] if li == 2 else hlon[li - 1]
                    ph = ppool.tile([128, 1024], f32, tag="ph", name=f"ph{li}_l")
                    nc.tensor.matmul(
                        out=ph[0:64, 0:BLK],
                        lhsT=w_sb[0:64, wc : wc + 64],
                        rhs=prev[:, :],
                        start=True,
                        stop=True,
                    )
                    hl = hpool.tile(
                        [64, BLK], f16, tag=f"hl{li}", name=f"h{li}_l", bufs=1
                    )
                    act(hl[:, :], ph[0:64, 0:BLK], bias_sb[bap_rows, li - 1 : li], True)
                    hlon[li] = hl
                    return
                ph = ppool.tile([128, 1024], f32, tag="ph", name=f"ph{li}_{u}")
                w = len(dbl_pairs(u)) * BLK
                for k, p in enumerate(dbl_pairs(u)):
                    co = k * BLK
                    rhs = h1s[p][:, :] if li == 2 else hdbl[li - 1][u][:, co : co + BLK]
                    nc.tensor.matmul(
                        out=ph[:, co : co + BLK],
                        lhsT=w_sb[:, wc : wc + 128],
                        rhs=rhs,
                        start=True,
                        stop=True,
                    )
                h = hpool.tile([128, w], f16, tag=f"h{li}_{u}", name=f"h{li}_{u}", bufs=1)
                act(h[:, :], ph[:, 0:w], bias_sb[:, li - 1 : li], True)
                hdbl[li][u] = h

            def emit_s6(u):
                # L6 [64 -> 18]: both pairs of group u side by side in one
                # [64, 1024] PSUM tile (pair k at cols k*512, block even rows
                # 0:18, block odd 32:50); u == -1 = lone block
                if u == -1:
                    po = popool.tile([64, 1024], f32, tag="po", name="po_l")
                    nc.tensor.matmul(
                        out=po[:, 0:BLK],
                        lhsT=w_sb[0:64, 640:704],
                        rhs=hlon[5][:, :],
                        start=True,
                        stop=True,
                    )
                    o = opool.tile([64, BLK], f16, tag="og", name="o_l")
                    act(o[:, :], po[:, 0:BLK], bias_sb[0:64, 5:6], False)
                    nc.gpsimd.dma_start(
                        out=yt[:, ndbl * 2 * BLK : ndbl * 2 * BLK + BLK], in_=o[:, :]
                    )
                    return
                pairs = dbl_pairs(u)
                w = len(pairs) * BLK
                po = popool.tile([64, 1024], f32, tag="po", name=f"po_{u}")
                for k, q in enumerate(pairs):
                    nc.tensor.matmul(
                        out=po[:, k * BLK : (k + 1) * BLK],
                        lhsT=w_sb[:, 640:704],
                        rhs=hdbl[5][u][:, k * BLK : (k + 1) * BLK],
                        start=True,
                        stop=True,
                    )
                o = opool.tile([64, w], f16, tag="og", name=f"o_{u}")
                act(o[:, :], po[:, 0:w], bias_sb[0:64, 5:6], False)
                nc.gpsimd.dma_start(
                    out=yt[:, u * 2 * BLK : u * 2 * BLK + w], in_=o[:, :]
                )

            # ---- depth-6 software pipeline: units in x-arrival order, lone
            # last. Stage s of unit i goes in wave i+s-1; within a wave the
            # rotated order S4,S5,S6,S3,S2,S1 keeps >=2 stages between every
            # producer act and its consumer matmul (act latency slack) while
            # the x-gated S1 stays last so it never blocks ready work.
            units = list(range(ndbl)) + ([-1] if lone else [])
            nunits = len(units)
            lag = {1: 0, 2: 1, 3: 2, 4: 3, 5: 4, 6: 5}

            def emit_stage(s, u):
                if s == 1:
                    emit_s1(u)
                elif s == 6:
                    emit_s6(u)
                else:
                    emit_mid(s, u)

            nfill = 0
            for wave in range(nunits + lag[6]):
                for s in (4, 5, 6, 3, 2, 1):
                    i = wave - lag[s]
                    if 0 <= i < nunits:
                        emit_stage(s, units[i])
                if wave < 2:
                    # keep-warm fillers bridge early x-DMA waits so the HAM
                    # clock gate never drops the PE back to 1.2 GHz
                    for _ in range(2 - wave):
                        pw = ppool.tile([128, 1024], f32, tag="ph", name=f"fl{nfill}")
                        nfill += 1
                        nc.tensor.matmul(
                            out=pw[0:64, 0:BLK],
                            lhsT=warm_src[0:1, 0:64],
                            rhs=warm_src[0:1, :],
                            start=True,
                            stop=True,
                        )

    nc.compile()
    return nc


def _get_program(nb: int):
    if nb not in _PROGRAM_CACHE:
        _PROGRAM_CACHE[nb] = _build_program(nb)
    return _PROGRAM_CACHE[nb]


def _prepare(state, rm_state, W1, b1, W2, b2, W3, b3, W4, b4, W5, b5, W6, b6):
    state = np.ascontiguousarray(np.asarray(state, dtype=np.float32))
    rm = np.asarray(rm_state).reshape(-1).astype(np.int64)
    Ws = [np.asarray(w, dtype=np.float32) for w in (W1, W2, W3, W4, W5, W6)]
    bs = [np.asarray(b, dtype=np.float32) for b in (b1, b2, b3, b4, b5, b6)]
    B = state.shape[0]
    X = state.reshape(B, D)

    # ---- host-side routing: all rows of expert k go to core k
    order = np.argsort(rm, kind="stable")
    counts = np.bincount(rm, minlength=E)
    nb = max(2, math.ceil(counts.max() / BLK))
    C = nb * BLK
    npair = nb // 2
    lone = nb % 2
    ndbl = (npair + 1) // 2
    ngrp = ndbl + lone
    csum = np.zeros(E, dtype=np.int64)
    csum[1:] = np.cumsum(counts)[:-1]
    sorted_expert = rm[order]
    pos_sorted = sorted_expert * C + (np.arange(B) - csum[sorted_expert])

    Xp = np.zeros((E * C, D), np.float16)
    Xp[pos_sorted] = X[order].astype(np.float16)

    W16 = [w.astype(np.float16) for w in Ws]

    in_maps = []
    for core in range(E):
        xt = Xp[core * C : (core + 1) * C].T  # [D, C] fp16 view
        # pairs: interleave the two 128-row halves per pair -> [128, 2048]
        parts = [
            xt[:, : npair * 1024]
            .reshape(2, 128, npair, 2 * BLK)
            .transpose(1, 2, 0, 3)
            .reshape(128, npair * 4 * BLK)
        ]
        if lone:
            xlh = xt[:, npair * 1024 :].reshape(2, 128, BLK)
            parts.append(xlh[0])
            parts.append(xlh[1])
        xint = np.ascontiguousarray(np.concatenate(parts, axis=1))

        wh = np.zeros((128, WCOLS), np.float16)
        wh[:, 0:H] = W16[0][core, 0:128, :]
        wh[:, H : 2 * H] = W16[0][core, 128:256, :]
        for li in range(4):
            wc = 128 + li * 128
            wh[0:64, wc : wc + H] = W16[li + 1][core]
            wh[64:128, wc + H : wc + 128] = W16[li + 1][core]
        wh[0:64, 640 : 640 + A] = W16[5][core]
        wh[64:128, 672 : 672 + A] = W16[5][core]

        bh = np.zeros((128, 6), np.float32)
        for li in range(5):
            bh[0:64, li] = bs[li][core]
            bh[64:128, li] = bs[li][core]
        for r0 in (0, 32, 64, 96):
            bh[r0 : r0 + A, 5] = bs[5][core]

        in_maps.append({"xall": xint, "wt": wh, "bias": bh})

    meta = dict(
        B=B,
        C=C,
        nb=nb,
        npair=npair,
        lone=lone,
        ndbl=ndbl,
        ngrp=ngrp,
        order=order,
        pos_sorted=pos_sorted,
    )
    return in_maps, meta


def _finalize(results, meta):
    """results: list (per core) of dicts with 'yt' [64, ...] fp16; pair q at
    cols q*512 (block even rows 0:18, block odd rows 32:50), lone block at
    cols npair*512."""
    B, C, nb, npair, lone, ndbl = (
        meta[k] for k in ("B", "C", "nb", "npair", "lone", "ndbl")
    )
    Yp = np.zeros((E * C, A), np.float32)
    for core in range(E):
        ytc = results[core]["yt"].astype(np.float32)
        for q in range(npair):
            cols = slice(q * BLK, (q + 1) * BLK)
            dst = core * C + 2 * q * BLK
            Yp[dst : dst + BLK] = ytc[0:A, cols].T
            Yp[dst + BLK : dst + 2 * BLK] = ytc[32 : 32 + A, cols].T
        if lone:
            cols = slice(npair * BLK, (npair + 1) * BLK)
            dst = core * C + (nb - 1) * BLK
            Yp[dst : dst + BLK] = ytc[0:A, cols].T

    y = np.zeros((B, A), np.float32)
    y[meta["order"]] = Yp[meta["pos_sorted"]]
    return y


def kernel(state, rm_state, W1, b1, W2, b2, W3, b3, W4, b4, W5, b5, W6, b6):
    global LAST_RESULTS
    from concourse.bass_utils import run_bass_kernel_spmd

    in_maps, meta = _prepare(
        state, rm_state, W1, b1, W2, b2, W3, b3, W4, b4, W5, b5, W6, b6
    )
    nc = _get_program(meta["nb"])
    trace = bool(os.environ.get("KERNEL_TRACE"))
    res = run_bass_kernel_spmd(nc, in_maps, core_ids=list(range(NCORES)), trace=trace)
    LAST_RESULTS = res
    return _finalize(res.results, meta)
